# revision 4
# baseline (speedup 1.0000x reference)
"""Trainium2 Bass kernel for nn_Block_19473381720396 (gnn_message_passing).

v3 design — tier-1 identity layout + o-major LN:

  host: per core, edges partitioned by out-owner. Per o-block (128 outputs):
  each output's first T1 edges sit at (tile t=rank, partition p=o&127) so the
  segment-sum over tier-1 tiles is plain PSUM accumulation with a constant
  identity stationary operand (no one-hot). Overflow edges go to T2 shared
  one-hot tiles per block. Streams fg/wg laid [NBLK, 128, TT*C].

  device phase A per block: DMA fg/wg; DVE mult -> ct; tiny is_eq for tier-2
  sel; T1 identity matmuls + T2 one-hot matmuls + 1 bias matmul accumulate
  x into psA [128o, 96c]; tensor_tensor_reduce gives xc (sbuf bf16) + row
  sum s1; gpsimd stt gives sq sum s2. Stats are batched per group of G
  blocks: negmu/var/std/rstd as [128, G] ops (ACT Sqrt once per group).
  xln = (xc + negmu)*rstd via one 2-scalar tensor_scalar; PE transpose to
  c-major xlnT.

  device phase B per slice (512 outputs): 96->384 gelu 384->96 MLP on
  xlnT [96,512], residual add from host-transposed feats, f32 out [C, NOP].
"""
import sys

for _p in ("/opt/trn_rl_repo",):
    if _p not in sys.path:
        sys.path.insert(0, _p)

import numpy as np
import ml_dtypes

import concourse.bacc as bacc
import concourse.bass as bass
import concourse.mybir as mybir
import concourse.tile as tile
from concourse.bass_utils import run_bass_kernel_spmd

# ---------------- problem constants (hardcoded) ----------------
NV = 200000        # voxels
C = 96             # channels
KV = 343           # kernel offsets
NCORE = 8
VPC = NV // NCORE  # 25000 voxels per core
NBLK = 196         # o-blocks of 128 (25088 padded o rows per core)
NOP = NBLK * 128   # 25088
NSL = NBLK // 4    # 49 phase-B slices of 512 outputs
GBLK = 28          # blocks per stats group
NGRP = NBLK // GBLK  # 7
EPS = 1e-6

# ---- tunables ----
T1 = 14            # tier-1 identity tiles (adaptive, recomputed per instance)
T2 = 4             # tier-2 one-hot tiles
STREAM_FP8 = False  # fg/wg streams as fp8 in HBM
STREAM_CAST = False  # load fp8 streams via gpsimd cast-DMA into bf16 SBUF
WSCALE = 32.0      # w_dw scale (power of 2; LN is scale-invariant)

SIM_MODE = False   # replace Gelu with Identity (CoreSim lacks Gelu)
DBG_NO_MLP = False   # phase B: skip MLP, out = fo only
DBG_NO_A2 = False    # skip xln/transpose/copy (phase A2)
DBG_A1 = 0           # 0=all, 1=no stats, 2=no stt/ttr, 3=no matmul, 4=no dve

TRACE = False
LAST_RESULT = None

_BF16 = ml_dtypes.bfloat16
_FP8 = ml_dtypes.float8_e4m3fn


# ---------------- host-side prep ----------------

def _choose_tiers(all_deg_blocks):
    """all_deg_blocks: [ncore*NBLK, 128] per-output degrees.
    Pick (T1, T2) minimizing T1+T2 with T2 = max block overflow tiles."""
    best = None
    for t1 in range(10, 26):
        over = np.maximum(all_deg_blocks - t1, 0).sum(axis=1)  # per block
        t2 = int(np.ceil(over.max() / 128.0)) if over.max() > 0 else 1
        t2 = max(t2, 1)
        tot = t1 + t2
        if best is None or tot < best[0] or (tot == best[0] and t1 > best[1]):
            best = (tot, t1, t2)
    return best[1], best[2]


def _pack_core(o, i, k, feats_q, wq, dtype):
    """Pack one core's edges into tier-1/tier-2 slot layout.

    o: local out idx [E], i: global in idx, k: kernel idx.
    Returns fg [NBLK,128,TT*C], wg same, ol2 [NBLK,128,T2] bf16.
    """
    TT = T1 + T2
    E = len(o)
    order = np.argsort(o, kind="stable")
    os_, is_, ks_ = o[order], i[order], k[order]
    # rank of each edge within its output
    deg = np.bincount(os_, minlength=NOP)
    starts = np.zeros(NOP, np.int64)
    starts[1:] = np.cumsum(deg)[:-1]
    rank = np.arange(E, dtype=np.int64) - starts[os_]
    blk = os_ >> 7
    p = os_ & 127

    nslot = NBLK * 128 * TT
    # slot index layout [NBLK, 128, TT]: slot = (blk*128 + p)*TT + t
    t1m = rank < T1
    slot1 = (blk[t1m] * 128 + p[t1m]) * TT + rank[t1m]

    # tier-2: per block sequential positions
    t2m = ~t1m
    blk2 = blk[t2m]
    ord2 = np.argsort(blk2, kind="stable")
    cnt2 = np.bincount(blk2, minlength=NBLK)
    if cnt2.max() > T2 * 128:
        raise RuntimeError(f"tier2 overflow {cnt2.max()} > {T2*128}")
    st2 = np.zeros(NBLK, np.int64)
    st2[1:] = np.cumsum(cnt2)[:-1]
    pos2 = np.arange(len(blk2), dtype=np.int64) - np.repeat(st2, cnt2)
    # position j*128+pp -> tile T1+j, partition pp
    j2 = pos2 >> 7
    pp2 = pos2 & 127
    e2 = np.nonzero(t2m)[0][ord2]
    slot2 = (blk2[ord2] * 128 + pp2) * TT + (T1 + j2)

    fg = np.zeros((nslot, C), _BF16)
    wg = np.zeros((nslot, C), _FP8)
    fg[slot1] = feats_q[is_[t1m]]
    wg[slot1] = wq[ks_[t1m]]
    fg[slot2] = feats_q[is_[e2]]
    wg[slot2] = wq[ks_[e2]]

    ol2 = np.full((NBLK, 128, T2), 255.0, np.float32)
    ol2[blk2[ord2], pp2, j2] = p[t2m][ord2]

    # quad-block major: fg bf16, wg fp8 (ACT upconverts on device)
    fg = np.ascontiguousarray(
        fg.reshape(NSL, 4, 128, TT * C).transpose(0, 2, 1, 3)
    ).reshape(NSL, 128, 4 * TT * C)
    wg = np.ascontiguousarray(
        wg.reshape(NSL, 4, 128, TT * C).transpose(0, 2, 1, 3)
    ).reshape(NSL, 128, 4 * TT * C)
    return fg, wg, ol2.astype(_BF16)


def _prep(inputs):
    global T1, T2
    feats = np.asarray(inputs["feats"], np.float32)
    w_dw = np.asarray(inputs["w_dw"], np.float32)
    b_dw = np.asarray(inputs["b_dw"], np.float32)
    ln_w = np.asarray(inputs["ln_w"], np.float32)
    ln_b = np.asarray(inputs["ln_b"], np.float32)
    w1 = np.asarray(inputs["w1"], np.float32)
    b1 = np.asarray(inputs["b1"], np.float32)
    w2 = np.asarray(inputs["w2"], np.float32)
    b2 = np.asarray(inputs["b2"], np.float32)
    in_idx = np.asarray(inputs["in_idx"], np.int64)
    out_idx = np.asarray(inputs["out_idx"], np.int64)
    kernel_idx = np.asarray(inputs["kernel_idx"], np.int64)

    feats_q = feats.astype(_BF16)
    wq = (w_dw * WSCALE).astype(_FP8)

    owner = out_idx // VPC
    # adaptive tier sizing over all cores
    degs = []
    for c in range(NCORE):
        o_c = out_idx[owner == c] - c * VPC
        deg = np.bincount(o_c, minlength=NOP)
        degs.append(deg.reshape(NBLK, 128))
    T1, T2 = _choose_tiers(np.concatenate(degs, axis=0))

    # constants
    ident = np.eye(128, dtype=np.float32).astype(_BF16)
    # iota2 o-major j-inner [128, 128, T2]: value = o index (2x-mode layout)
    iota2 = np.ascontiguousarray(
        np.broadcast_to(np.arange(128, dtype=np.float32)[None, :, None],
                        (128, 128, T2))).reshape(128, 128 * T2).astype(_BF16)
    onec = np.ones((1, 128), np.float32).astype(_BF16)
    bdwr = np.ascontiguousarray(
        np.broadcast_to(b_dw * WSCALE, (128, C)), np.float32)
    w1p = (ln_w[:, None] * w1).astype(_BF16)                    # [96, 384]
    b1p = np.ascontiguousarray(
        (b1 + ln_b @ w1).reshape(3, 128).T, np.float32)         # [128, 3]
    w2p = np.ascontiguousarray(
        w2.reshape(3, 128, C).transpose(1, 0, 2)).astype(_BF16)  # [128, 3, 96]
    b2c = np.ascontiguousarray(b2.reshape(C, 1), np.float32)

    maps = []
    for c in range(NCORE):
        m_ = np.nonzero(owner == c)[0]
        fg, wg8, ol2 = _pack_core(out_idx[m_] - c * VPC, in_idx[m_],
                                  kernel_idx[m_], feats_q, wq, None)
        # group olb: [NGRP, 128, GBLK*T2]
        olg = np.ascontiguousarray(
            ol2.reshape(NGRP, GBLK, 128, T2).transpose(0, 2, 1, 3)
        ).reshape(NGRP, 128, GBLK * T2)
        fown = np.zeros((C, NOP), _BF16)
        fown[:, :VPC] = feats[c * VPC:(c + 1) * VPC].T.astype(_BF16)
        maps.append({
            "fgq": fg, "wgq8": wg8, "olg": olg,
            "ident": ident, "iota2": iota2, "onec": onec, "bdwr": bdwr,
            "w1p": w1p, "b1p": b1p, "w2p": w2p, "b2c": b2c, "fown": fown,
        })
    return maps


# ---------------- device program ----------------

def _build():
    TT = T1 + T2
    nc = bacc.Bacc("TRN2", target_bir_lowering=False, debug=False)
    dt = mybir.dt
    sdt = dt.float8e4 if STREAM_FP8 else dt.bfloat16
    fgq = nc.dram_tensor("fgq", [NSL, 128, 4 * TT * C], dt.bfloat16,
                         kind="ExternalInput")
    wgq8 = nc.dram_tensor("wgq8", [NSL, 128, 4 * TT * C], dt.float8e4,
                          kind="ExternalInput")
    olg = nc.dram_tensor("olg", [NGRP, 128, GBLK * T2], dt.bfloat16,
                         kind="ExternalInput")
    identt = nc.dram_tensor("ident", [128, 128], dt.bfloat16, kind="ExternalInput")
    iota2t = nc.dram_tensor("iota2", [128, 128 * T2], dt.bfloat16,
                            kind="ExternalInput")
    onect = nc.dram_tensor("onec", [1, 128], dt.bfloat16, kind="ExternalInput")
    bdwrt = nc.dram_tensor("bdwr", [128, C], dt.float32, kind="ExternalInput")
    w1pt = nc.dram_tensor("w1p", [C, 4 * C], dt.bfloat16, kind="ExternalInput")
    b1pt = nc.dram_tensor("b1p", [128, 3], dt.float32, kind="ExternalInput")
    w2pt = nc.dram_tensor("w2p", [128, 3, C], dt.bfloat16, kind="ExternalInput")
    b2ct = nc.dram_tensor("b2c", [C, 1], dt.float32, kind="ExternalInput")
    fown = nc.dram_tensor("fown", [C, NOP], dt.bfloat16, kind="ExternalInput")
    outp = nc.dram_tensor("outp", [C, NOP], dt.float32, kind="ExternalOutput")

    AL = mybir.AluOpType
    AF = mybir.ActivationFunctionType
    EPS_S = WSCALE * WSCALE * EPS

    with tile.TileContext(nc) as tc:
        with tc.tile_pool(name="const", bufs=1) as cpool, \
             tc.tile_pool(name="io", bufs=3) as io, \
             tc.tile_pool(name="work", bufs=2) as wk, \
             tc.tile_pool(name="grp", bufs=2) as gp, \
             tc.tile_pool(name="mlp", bufs=2) as mp:
            ident_t = cpool.tile([128, 128], dt.bfloat16)
            nc.sync.dma_start(out=ident_t[:], in_=identt[:])
            iota_t = cpool.tile([128, 128, T2], dt.bfloat16)
            nc.sync.dma_start(
                out=iota_t[:].rearrange("p a b -> p (a b)"), in_=iota2t[:])
            onec_t = cpool.tile([1, 128], dt.bfloat16)
            nc.sync.dma_start(out=onec_t[:], in_=onect[:])
            bdwr_t = cpool.tile([128, C], dt.float32)
            nc.sync.dma_start(out=bdwr_t[:], in_=bdwrt[:])
            w1_t = cpool.tile([C, 4 * C], dt.bfloat16)
            nc.sync.dma_start(out=w1_t[:], in_=w1pt[:])
            b1_t = cpool.tile([128, 3], dt.float32)
            nc.sync.dma_start(out=b1_t[:], in_=b1pt[:])
            w2_t = cpool.tile([128, 3, C], dt.bfloat16)
            nc.sync.dma_start(out=w2_t[:], in_=w2pt[:])
            b2_t = cpool.tile([C, 1], dt.float32)
            nc.sync.dma_start(out=b2_t[:], in_=b2ct[:])
            eps_t = cpool.tile([128, 1], dt.float32)
            nc.vector.memset(eps_t[:], EPS_S)

            with tc.tile_pool(name="psA", bufs=2, space="PSUM") as ppA, \
                 tc.tile_pool(name="psT", bufs=2, space="PSUM") as ppT, \
                 tc.tile_pool(name="psH", bufs=2, space="PSUM") as ppH, \
                 tc.tile_pool(name="psX", bufs=2, space="PSUM") as ppX:

                def emit_sums(pend, s1c, s2c, xcg):
                    """Deferred per-block xc/s1/s2 ops (lagged one quad)."""
                    for psA4, qi in pend:
                        for b4 in range(4):
                            bi = qi * 4 + b4
                            nc.vector.scalar_tensor_tensor(
                                out=xcg[:, bi, :], in0=psA4[:, b4, :],
                                scalar=1.0, in1=bdwr_t[:], op0=AL.mult,
                                op1=AL.add, accum_out=s1c[:, bi:bi + 1])
                            sq = wk.tile([128, C], dt.bfloat16, tag="sq")
                            nc.vector.scalar_tensor_tensor(
                                out=sq[:], in0=xcg[:, bi, :], scalar=1.0,
                                in1=xcg[:, bi, :], op0=AL.mult,
                                op1=AL.mult, accum_out=s2c[:, bi:bi + 1])
                    pend.clear()

                def emit_A1(g, olb, s1c, s2c, xcg):
                    pend = []
                    for qi in range(GBLK // 4):
                        q = g * (GBLK // 4) + qi
                        fgb = io.tile([128, 4, TT, C], dt.bfloat16,
                                      tag="fgb", bufs=4)
                        nc.sync.dma_start(
                            out=fgb[:].rearrange("p a t c -> p (a t c)"),
                            in_=fgq[q])
                        wgb8 = io.tile([128, 4, TT, C], dt.float8e4,
                                       tag="wgb8", bufs=4)
                        nc.sync.dma_start(
                            out=wgb8[:].rearrange("p a t c -> p (a t c)"),
                            in_=wgq8[q])
                        wgb = wk.tile([128, 4, TT, C], dt.bfloat16, tag="wgb")
                        nc.scalar.activation(
                            out=wgb[:].rearrange("p a t c -> p (a t c)"),
                            in_=wgb8[:].rearrange("p a t c -> p (a t c)"),
                            func=AF.Copy)
                        # sel quad [128, 4, 128, T2] (o-major j-inner, 2x)
                        sel = wk.tile([128, 4, 128, T2], dt.bfloat16, tag="sel")
                        nc.vector.tensor_tensor(
                            out=sel[:],
                            in0=olb[:, qi * 4:qi * 4 + 4, :].rearrange(
                                "p a (o t) -> p a o t", o=1).to_broadcast(
                                    [128, 4, 128, T2]),
                            in1=iota_t[:].rearrange(
                                "p (a o) t -> p a o t", a=1).to_broadcast(
                                    [128, 4, 128, T2]),
                            op=AL.is_equal)
                        ct = wk.tile([128, 4, TT, C], dt.bfloat16, tag="ct")
                        nc.vector.tensor_tensor(
                            out=ct[:].rearrange("p a t c -> p (a t c)"),
                            in0=fgb[:].rearrange("p a t c -> p (a t c)"),
                            in1=wgb[:].rearrange("p a t c -> p (a t c)"),
                            op=AL.mult)
                        psA4 = ppA.tile([128, 4, C], dt.float32, tag="psA")
                        # tier-1 first (needs only ct), tier-2 after (needs sel)
                        for b4 in range(4):
                            for t in range(T1):
                                nc.tensor.matmul(
                                    out=psA4[:, b4, :], lhsT=ident_t[:],
                                    rhs=ct[:, b4, t, :],
                                    start=(b4 == 0 and t == 0), stop=False,
                                    skip_group_check=True)
                        for b4 in range(4):
                            for j in range(T2):
                                nc.tensor.matmul(
                                    out=psA4[:, b4, :], lhsT=sel[:, b4, :, j],
                                    rhs=ct[:, b4, T1 + j, :], start=False,
                                    stop=(b4 == 3 and j == T2 - 1),
                                    skip_group_check=True)
                        if qi > 0:
                            emit_sums(pend, s1c, s2c, xcg)
                        pend.append((psA4, qi))
                    emit_sums(pend, s1c, s2c, xcg)

                def emit_stats(st):
                    s1c, s2c = st["s1c"], st["s2c"]
                    negmu = gp.tile([128, GBLK], dt.float32, tag="negmu")
                    nc.vector.tensor_scalar(out=negmu[:], in0=s1c[:],
                                            scalar1=-1.0 / C, scalar2=None,
                                            op0=AL.mult)
                    var = gp.tile([128, GBLK], dt.float32, tag="var")
                    nc.vector.scalar_tensor_tensor(
                        out=var[:], in0=negmu[:], scalar=0.0, in1=negmu[:],
                        op0=AL.add, op1=AL.mult)
                    nc.vector.scalar_tensor_tensor(
                        out=var[:], in0=s2c[:], scalar=1.0 / C, in1=var[:],
                        op0=AL.mult, op1=AL.subtract)
                    stdg = gp.tile([128, GBLK], dt.float32, tag="stdg")
                    nc.scalar.activation(out=stdg[:], in_=var[:], func=AF.Sqrt,
                                         bias=eps_t[:], scale=1.0)
                    rstd = gp.tile([128, GBLK], dt.float32, tag="rstd")
                    nc.vector.reciprocal(rstd[:], stdg[:])
                    st["negmu"], st["rstd"] = negmu, rstd

                def emit_A2(st):
                    xcg, negmu, rstd = st["xcg"], st["negmu"], st["rstd"]
                    xlT = gp.tile([C, GBLK, 128], dt.bfloat16, tag="xlT")
                    for bi in range(0 if DBG_NO_A2 else GBLK):
                        xln = wk.tile([128, C], dt.bfloat16, tag="xln")
                        nc.vector.tensor_scalar(
                            out=xln[:], in0=xcg[:, bi, :],
                            scalar1=negmu[:, bi:bi + 1],
                            scalar2=rstd[:, bi:bi + 1],
                            op0=AL.add, op1=AL.mult)
                        psT = ppT.tile([C, 128], dt.bfloat16, tag="psT")
                        nc.tensor.transpose(psT[:], xln[:], ident_t[:])
                        nc.scalar.activation(out=xlT[:, bi, :], in_=psT[:],
                                             func=AF.Copy)
                    st["xlT"] = xlT

                def emit_B(g, xlT):
                    for si in range(GBLK // 4):
                        s = g * (GBLK // 4) + si
                        xv = xlT[:, si * 4:(si + 1) * 4, :].rearrange(
                            "c a b -> c (a b)")
                        fo = io.tile([C, 512], dt.bfloat16, tag="fo")
                        nc.scalar.dma_start(
                            out=fo[:], in_=fown[:, s * 512:(s + 1) * 512])
                        if DBG_NO_MLP:
                            nc.scalar.dma_start(
                                out=outp[:, s * 512:(s + 1) * 512], in_=fo[:])
                            continue
                        hts = []
                        for k in range(3):
                            psh = ppH.tile([128, 512], dt.float32, tag="psh")
                            nc.tensor.matmul(
                                out=psh[:],
                                lhsT=w1_t[:, k * 128:(k + 1) * 128],
                                rhs=xv, start=True, stop=True)
                            ht = mp.tile([128, 512], dt.bfloat16, tag=f"ht{k}")
                            nc.scalar.activation(out=ht[:], in_=psh[:],
                                                 func=(AF.Identity if SIM_MODE
                                                       else AF.Gelu),
                                                 bias=b1_t[:, k:k + 1],
                                                 scale=1.0)
                            hts.append(ht)
                        psx = ppX.tile([C, 512], dt.float32, tag="psx")
                        for k in range(3):
                            nc.tensor.matmul(out=psx[:], lhsT=w2_t[:, k, :],
                                             rhs=hts[k][:],
                                             start=(k == 0), stop=(k == 2))
                        o32 = mp.tile([C, 512], dt.float32, tag="o32")
                        nc.vector.scalar_tensor_tensor(
                            out=o32[:], in0=psx[:], scalar=b2_t[:],
                            in1=fo[:], op0=AL.add, op1=AL.add)
                        nc.scalar.dma_start(out=outp[:, s * 512:(s + 1) * 512],
                                            in_=o32[:])

                # pipelined emission: phase B lags one group
                prev = None
                for g in range(NGRP):
                    olb = io.tile([128, GBLK, T2], dt.bfloat16, tag="olb")
                    nc.sync.dma_start(
                        out=olb[:].rearrange("p a b -> p (a b)"), in_=olg[g])
                    st = {
                        "s1c": gp.tile([128, GBLK], dt.float32, tag="s1c",
                                       name="s1c"),
                        "s2c": gp.tile([128, GBLK], dt.float32, tag="s2c",
                                       name="s2c"),
                        "xcg": gp.tile([128, GBLK, C], dt.bfloat16, tag="xcg",
                                       name="xcg"),
                    }
                    emit_A1(g, olb, st["s1c"], st["s2c"], st["xcg"])
                    emit_stats(st)
                    if prev is not None:
                        emit_B(g - 1, prev["xlT"])
                    emit_A2(st)
                    prev = st
                emit_B(NGRP - 1, prev["xlT"])
    nc.compile()
    return nc


# ---------------- public entry ----------------

_NC_CACHE = {}


def kernel(**inputs):
    global LAST_RESULT
    import os
    ncores_run = int(os.environ.get("KERNEL_NCORES", str(NCORE)))
    maps = _prep(inputs)
    key = (T1, T2, STREAM_FP8, STREAM_CAST)
    if key not in _NC_CACHE:
        _NC_CACHE[key] = _build()
    nc = _NC_CACHE[key]
    kw = {}
    if TRACE:
        kw.update(trace=True)
    res = run_bass_kernel_spmd(nc, maps[:ncores_run],
                               core_ids=list(range(ncores_run)), **kw)
    LAST_RESULT = res
    out = np.zeros((NV, C), np.float32)
    for c in range(ncores_run):
        out[c * VPC:(c + 1) * VPC] = res.results[c]["outp"][:, :VPC].T
    return out
